# revision 3
# baseline (speedup 1.0000x reference)
"""Multi-head attention (strictly-upper-triangular mask variant) on 8 TRN2 cores.

Reference math (B=4, S=2048, D=512, H=8, A=64):
    q/k/v = per-head projections of query/key/value           [B,H,S,A]
    scores = q @ k^T / sqrt(A), masked where k <= q (lower triangle incl diag
    masked to -1e9 -> softmax attends strictly to FUTURE positions)
    out = concat_heads(softmax(scores) @ v) @ Wo + bo         [B,S,D]

Sharding: 8 cores = 4 batches x 2 interleaved q-tile sets.  Core c handles
batch b=c//2, q-tiles g = 2*i + (c%2) for i in 0..7 (128 rows each).  Every
core computes all 8 heads for its 1024 query rows; no collectives needed --
the host gather is a pure row-interleave concat.

v2 pipeline (vs v1): PE was the bottleneck at ~74% busy with ~50us of HAM
cold-clock penalty from pipeline stalls plus a 23us input-DMA serialization
at the head.  Fixes:
  - inputs DMA'd in first-use order and in chunks, so Q-projection starts
    ~5us in instead of waiting for the full 7MB load;
  - PSUM split into dedicated pools: scores (1x[128,2048] holding BOTH heads
    of a pair), AV accumulators (2x[128,512]), projections (1x[128,1024]),
    so the Tile scheduler can interleave projection / score / AV matmuls and
    the PE never idles long enough to re-throttle;
  - the two heads of a pair issue score matmuls on disjoint PE row groups
    (partitions 0-63 / 64-127) back to back, so they execute concurrently;
  - masking is applied by an accumulating matmul (identity stationary,
    -1e9 additive mask moving) into the score PSUM *before* exp, removing
    the 128 DVE mask-multiplies from the score->AV critical chain;
  - score strips are packed into 9 exactly-1024-wide bins per head; one
    2048-wide exp per bin covers both heads (fewer ACT fixed overheads);
  - softmax work is software-pipelined across head pairs: while ACT
    exponentiates pair p, the PE runs AV for pair p-1's second head, AV for
    pair p's first head, and the projections of pair p+1.
"""

import numpy as np
import ml_dtypes

B, S, D, H, A = 4, 2048, 512, 8, 64
P = 128
NQ = 1024          # q rows per core
NQT = 8            # q tiles per core
NKC = 16           # k chunks
NPAIR = 4          # head pairs
BF = ml_dtypes.bfloat16

# strip widths for the transposed-score layout (strip kc holds the q-tiles
# that need k-chunk kc; width 128*(kc//2+1))
WKC = [P * (kc // 2 + 1) for kc in range(NKC)]

# 9 bins of exactly 1024 columns each: strips packed two-per-bin (or the
# 1024-wide strips alone)
BINS = [(14,), (15,), (12, 0), (13, 1), (10, 2), (11, 3), (8, 4), (9, 5), (6, 7)]
NBIN = len(BINS)
STRIP_BIN = {}   # kc -> (bin index, offset within bin)
for bi, strips in enumerate(BINS):
    off = 0
    for kc in strips:
        STRIP_BIN[kc] = (bi, off)
        off += WKC[kc]
    assert off == 1024

_cache = {}


def _split512(a, b):
    """Split [a,b) at multiples of 512 (PSUM bank boundaries)."""
    out = []
    while a < b:
        nxt = min(b, (a // 512 + 1) * 512)
        out.append((a, nxt))
        a = nxt
    return out


def _build():
    if "nc" in _cache:
        return _cache["nc"]

    import concourse.bacc as bacc
    import concourse.mybir as mybir
    import concourse.tile as tile

    F32 = mybir.dt.float32
    BF16 = mybir.dt.bfloat16
    MULT = mybir.AluOpType.mult
    EXP = mybir.ActivationFunctionType.Exp

    nc = bacc.Bacc("TRN2", target_bir_lowering=False, debug=False, num_devices=8)

    qT8_d = nc.dram_tensor("qT8", [P, 4 * NQ], BF16, kind="ExternalInput")
    kT_d = nc.dram_tensor("kT", [P, 4 * S], BF16, kind="ExternalInput")
    vT_d = nc.dram_tensor("vT", [P, 4 * S], BF16, kind="ExternalInput")
    wq_d = nc.dram_tensor("wq", [P, 2048], BF16, kind="ExternalInput")
    wk_d = nc.dram_tensor("wk", [P, 2048], BF16, kind="ExternalInput")
    wv_d = nc.dram_tensor("wv", [P, 2048], BF16, kind="ExternalInput")
    wo_d = nc.dram_tensor("wo", [P, 2048], BF16, kind="ExternalInput")
    bq_d = nc.dram_tensor("bq8", [P, 4], F32, kind="ExternalInput")
    bk_d = nc.dram_tensor("bk", [P, 4], F32, kind="ExternalInput")
    bv_d = nc.dram_tensor("bv", [P, 4], F32, kind="ExternalInput")
    bo_d = nc.dram_tensor("bo", [1, D], BF16, kind="ExternalInput")
    ones_d = nc.dram_tensor("ones1", [1, P], BF16, kind="ExternalInput")
    id_d = nc.dram_tensor("ident", [P, P], BF16, kind="ExternalInput")
    ma_d = nc.dram_tensor("maskA", [P, P], BF16, kind="ExternalInput")
    mb_d = nc.dram_tensor("maskB", [P, P], BF16, kind="ExternalInput")
    out_d = nc.dram_tensor("out", [NQ, D], F32, kind="ExternalOutput")

    with tile.TileContext(nc) as tc:
        with (
            tc.tile_pool(name="cst", bufs=1) as cst,
            tc.tile_pool(name="act", bufs=1) as act,
            tc.tile_pool(name="vtt", bufs=2) as vtt,
            tc.tile_pool(name="ptp", bufs=14) as ptp,
            tc.tile_pool(name="rcp", bufs=3) as rcp,
            tc.tile_pool(name="ost", bufs=2) as ost,
            tc.tile_pool(name="scp", bufs=1, space="PSUM") as scp,
            tc.tile_pool(name="avp", bufs=2, space="PSUM") as avp,
            tc.tile_pool(name="pjp", bufs=1, space="PSUM") as pjp,
        ):
            # ---- SBUF homes for the inputs ----
            qT8 = cst.tile([P, 4 * NQ], BF16, tag="qT8")
            kT = cst.tile([P, 4 * S], BF16, tag="kT")
            vT = cst.tile([P, 4 * S], BF16, tag="vT")
            wq = cst.tile([P, 2048], BF16, tag="wq")
            wk = cst.tile([P, 2048], BF16, tag="wk")
            wv = cst.tile([P, 2048], BF16, tag="wv")
            wo = cst.tile([P, 2048], BF16, tag="wo")
            bq = cst.tile([P, 4], F32, tag="bq")
            bk = cst.tile([P, 4], F32, tag="bk")
            bv = cst.tile([P, 4], F32, tag="bv")
            bo = cst.tile([1, D], BF16, tag="bo")
            on1 = cst.tile([1, P], BF16, tag="on1")
            idt = cst.tile([P, P], BF16, tag="idt")
            mA = cst.tile([P, P], BF16, tag="mA")
            mB = cst.tile([P, P], BF16, tag="mB")
            warm = cst.tile([1, 2], BF16, tag="warm")

            # DMAs in first-use order: small consts + Q-side first so the
            # first projection matmuls start as early as possible, K-side
            # next, V-side, then the output projection weights.
            for t, d in [(idt, id_d), (mA, ma_d), (mB, mb_d), (bq, bq_d),
                         (bk, bk_d), (bv, bv_d), (on1, ones_d), (bo, bo_d)]:
                nc.sync.dma_start(t[:], d[:])
            # early dummy exp: forces the ACT table load off the critical path
            nc.scalar.activation(warm[:], idt[0:1, 0:2], EXP)
            nc.sync.dma_start(wq[:], wq_d[:])
            for ch in range(4):
                nc.sync.dma_start(qT8[:, NQ * ch:NQ * (ch + 1)],
                                  qT8_d[:, NQ * ch:NQ * (ch + 1)])
            nc.sync.dma_start(wk[:], wk_d[:])
            for ch in range(4):
                nc.sync.dma_start(kT[:, S * ch:S * (ch + 1)],
                                  kT_d[:, S * ch:S * (ch + 1)])
            nc.sync.dma_start(wv[:], wv_d[:])
            for ch in range(4):
                nc.sync.dma_start(vT[:, S * ch:S * (ch + 1)],
                                  vT_d[:, S * ch:S * (ch + 1)])
            nc.sync.dma_start(wo[:], wo_d[:])

            QT = [act.tile([P, NQ], BF16, tag=f"QT{p}", name=f"QT{p}") for p in range(NPAIR)]
            KT = [act.tile([P, S], BF16, tag=f"KT{p}", name=f"KT{p}") for p in range(NPAIR)]
            Vn = [act.tile([P, NKC * 192], BF16, tag=f"Vn{p}", name=f"Vn{p}") for p in range(NPAIR)]
            XT = [act.tile([P, NQ], BF16, tag=f"XT{c}", name=f"XT{c}") for c in range(4)]
            pts = [[None] * NBIN for _ in range(NPAIR)]  # pt tile APs

            # ---- projections for one head pair ----
            def project(p):
                # Q: one [128,1024] PSUM tile covering all 1024 q rows
                ps = pjp.tile([P, 1024], F32, tag="pj")
                for qh in range(2):
                    for ch in range(4):
                        nc.tensor.matmul(
                            ps[:, 512 * qh:512 * (qh + 1)],
                            wq[:, (4 * p + ch) * P:(4 * p + ch + 1) * P],
                            qT8[:, NQ * ch + 512 * qh: NQ * ch + 512 * (qh + 1)],
                            start=(ch == 0), stop=(ch == 3))
                nc.vector.tensor_scalar_add(QT[p][:], ps[:], bq[:, p:p + 1])
                # K: two [128,1024] tiles
                for s2 in range(2):
                    ps = pjp.tile([P, 1024], F32, tag="pj")
                    for sh in (2 * s2, 2 * s2 + 1):
                        for ch in range(4):
                            nc.tensor.matmul(
                                ps[:, 512 * (sh % 2):512 * (sh % 2 + 1)],
                                wk[:, (4 * p + ch) * P:(4 * p + ch + 1) * P],
                                kT[:, S * ch + 512 * sh: S * ch + 512 * (sh + 1)],
                                start=(ch == 0), stop=(ch == 3))
                    nc.vector.tensor_scalar_add(
                        KT[p][:, 1024 * s2:1024 * (s2 + 1)], ps[:], bk[:, p:p + 1])
                # V: two [128,1024] tiles -> VTp, then PE-transpose into Vn
                VTp = vtt.tile([P, S], BF16, tag="VT")
                for s2 in range(2):
                    ps = pjp.tile([P, 1024], F32, tag="pj")
                    for sh in (2 * s2, 2 * s2 + 1):
                        for ch in range(4):
                            nc.tensor.matmul(
                                ps[:, 512 * (sh % 2):512 * (sh % 2 + 1)],
                                wv[:, (4 * p + ch) * P:(4 * p + ch + 1) * P],
                                vT[:, S * ch + 512 * sh: S * ch + 512 * (sh + 1)],
                                start=(ch == 0), stop=(ch == 3))
                    nc.vector.tensor_scalar_add(
                        VTp[:, 1024 * s2:1024 * (s2 + 1)], ps[:], bv[:, p:p + 1])
                # V natural layout blocks [V_h0 | ones | V_h1] per k-chunk
                nc.gpsimd.memset(Vn[p][:], 1.0)
                for m in range(2):
                    vps = pjp.tile([P, 1024], BF16, tag="pj",
                                   padded_shape=[P, 2048])
                    for j in range(8):
                        nc.tensor.transpose(
                            vps[:, P * j:P * (j + 1)],
                            VTp[:, P * (8 * m + j):P * (8 * m + j + 1)], idt[:])
                    src = vps[:].rearrange("p (j c) -> p j c", c=P)
                    dst = Vn[p][:].rearrange("p (k c) -> p k c", c=192)
                    nc.vector.tensor_copy(dst[:, 8 * m:8 * m + 8, 0:64],
                                          src[:, :, 0:64])
                    nc.vector.tensor_copy(dst[:, 8 * m:8 * m + 8, 128:192],
                                          src[:, :, 64:128])

            # ---- scores + exp for one pair, one bin ----
            # PSUM `start=True` clears has_written for the WHOLE bank, so the
            # per-bank emission order is: additive mask matmuls first (deps
            # only on early consts -> ready first, lowest priority, so they
            # execute first; the sole start=True is on the first of them),
            # then the score matmuls, which overwrite fresh regions and
            # accumulate the -1e9 masks where they overlap.  Accumulation is
            # commutative and WAW overlaps order the rest, so any scheduler
            # interleaving is safe.
            def score_bin(p, bi):
                scps = scp.tile([P, 2048], F32, tag="sc")
                for hh in range(2):
                    hr = slice(64 * hh, 64 * hh + 64)
                    hc = 1024 * hh
                    banks = {0: [], 1: []}
                    for kc in BINS[bi]:
                        _, F = STRIP_BIN[kc]
                        W = WKC[kc]
                        banks[(F + W - P) // 512].append(("m", kc, F + W - P, F + W))
                    for kc in BINS[bi]:
                        _, F = STRIP_BIN[kc]
                        for (a0, a1) in _split512(F, F + WKC[kc]):
                            banks[a0 // 512].append(("s", kc, a0, a1))
                    for bank in (0, 1):
                        ops = banks[bank]
                        for n, (typ, kc, a0, a1) in enumerate(ops):
                            first, last = (n == 0), (n == len(ops) - 1)
                            if typ == "m":
                                nc.tensor.matmul(
                                    scps[:, hc + a0:hc + a1],
                                    idt[:], mA[:] if kc % 2 == 0 else mB[:],
                                    start=first, stop=last,
                                    skip_group_check=True)
                            else:
                                _, F = STRIP_BIN[kc]
                                nc.tensor.matmul(
                                    scps[:, hc + a0:hc + a1],
                                    KT[p][hr, P * kc:P * (kc + 1)],
                                    QT[p][hr, a0 - F:a1 - F],
                                    start=first, stop=last,
                                    skip_group_check=True)
                pt = ptp.tile([P, 2048], BF16, tag="pt", name=f"pt{p}_{bi}")
                nc.scalar.activation(pt[:], scps[:], EXP)
                pts[p][bi] = pt

            # ---- AV + normalize for one head ----
            def attention_av(h):
                p, hh = h // 2, h % 2
                hr = slice(64 * hh, 64 * hh + 64)
                orow = 0 if hh == 0 else 64
                drow = 64 - orow
                for b in range(2):
                    avb = avp.tile([P, 512], F32, tag="av")
                    kcs = [kc for kc in range(8 * b, NKC)]
                    for n, kc in enumerate(kcs):
                        bi, F = STRIP_BIN[kc]
                        w = min(WKC[kc], 512 * (b + 1)) - 512 * b
                        nc.tensor.matmul(
                            avb[:, 0:w],
                            Vn[p][:, 192 * kc + 64 * hh: 192 * kc + 64 * hh + 128],
                            pts[p][bi][:, 1024 * hh + F + 512 * b:
                                       1024 * hh + F + 512 * b + w],
                            start=(n == 0), stop=(n == len(kcs) - 1),
                            skip_group_check=True)
                    # approx-fast reciprocal cannot read PSUM; bounce the
                    # replicated denominators through SBUF
                    rec = rcp.tile([64, 1024], F32, tag="rec")
                    nc.vector.tensor_copy(rec[:, 0:512], avb[drow:drow + 64, :])
                    nc.vector.reciprocal_approx_fast(rec[:, 512:1024], rec[:, 0:512])
                    nc.vector.tensor_tensor(XT[p][hr, 512 * b:512 * (b + 1)],
                                            avb[orow:orow + 64, :], rec[:, 512:1024],
                                            MULT)

            # ---- pipelined emission ----
            project(0)
            for p in range(NPAIR):
                score_bin(p, 0)
                score_bin(p, 1)
                if p > 0:
                    attention_av(2 * (p - 1) + 1)   # h1 of previous pair
                for bi in range(2, NBIN):
                    score_bin(p, bi)
                attention_av(2 * p)                 # h0 of this pair
                if p < NPAIR - 1:
                    project(p + 1)
            attention_av(2 * (NPAIR - 1) + 1)

            # ---- output projection (two 512-wide tiles per PSUM tile) ----
            for t in range(4):
                po = pjp.tile([P, 1024], F32, tag="pj")
                for half in range(2):
                    i = 2 * t + half
                    col = 512 * half
                    for ch in range(4):
                        nc.tensor.matmul(po[:, col:col + 512],
                                         XT[ch][:, P * i:P * (i + 1)],
                                         wo[:, 512 * ch:512 * (ch + 1)],
                                         start=(ch == 0), stop=False)
                    nc.tensor.matmul(po[:, col:col + 512], on1[0:1, :], bo[0:1, :],
                                     start=False, stop=True)
                ob = ost.tile([P, 1024], F32, tag="ob")
                nc.vector.tensor_copy(ob[:], po[:])
                for half in range(2):
                    i = 2 * t + half
                    nc.sync.dma_start(out_d[P * i:P * (i + 1), :],
                                      ob[:, 512 * half:512 * (half + 1)])

    nc.compile()
    _cache["nc"] = nc
    return nc


def _host_prep(query, key, value, Wq, bq, Wk, bk, Wv, bv, Wo, bo):
    """Build the 8 per-core input maps (all device-side layouts)."""
    def stack_pairs(W):
        # [H,D,A] -> [128, 16*128]: col block (4p+ch) = rows 128ch of [Wq_2p|Wq_2p+1]
        blocks = []
        for p in range(NPAIR):
            Wp = np.concatenate([W[2 * p], W[2 * p + 1]], axis=1)  # [512, 128]
            for ch in range(4):
                blocks.append(Wp[P * ch:P * (ch + 1), :])
        return np.stack(blocks, 1).reshape(P, -1).astype(BF)  # [128,2048]

    wq_h, wk_h, wv_h = stack_pairs(Wq), stack_pairs(Wk), stack_pairs(Wv)
    wo_h = np.stack([Wo[P * ch:P * (ch + 1), :] for ch in range(4)], 1)
    wo_h = wo_h.reshape(P, -1).astype(BF)  # [128, 4*512]

    def stack_bias(b, scale=1.0):
        cols = [np.concatenate([b[2 * p], b[2 * p + 1]]) * scale for p in range(NPAIR)]
        return np.stack(cols, 1).astype(np.float32)  # [128, 4]

    bq_h = stack_bias(bq, 0.125)
    bk_h, bv_h = stack_bias(bk), stack_bias(bv)
    bo_h = bo[None, :].astype(BF)
    ones_h = np.ones((1, P), BF)
    id_h = np.eye(P, dtype=BF)
    kl = np.arange(P)[:, None]
    ql = np.arange(P)[None, :]
    # additive masks: -1e9 where the reference masks (k <= q) inside the
    # diagonal block.  For pair-1 cores the trailing q-slot of every even
    # strip is entirely in the past (k < q everywhere) -> mask all of it.
    neg = np.float32(-1e9)
    diag_mask = np.where(kl <= ql, neg, np.float32(0)).astype(BF)
    full_mask = np.full((P, P), neg, np.float32).astype(BF)
    zero_mask = np.zeros((P, P), BF)

    def chunked_T(x, scale=1.0):
        # [S', D] -> [128, 4*S'] with col block ch = rows 128ch of x.T
        xT = np.ascontiguousarray(x.T) * scale  # [512, S']
        return xT.reshape(4, P, -1).transpose(1, 0, 2).reshape(P, -1).astype(BF)

    in_maps = []
    for c in range(8):
        b, pair = c // 2, c % 2
        sel = np.concatenate(
            [np.arange(P * (2 * i + pair), P * (2 * i + pair) + P) for i in range(NQT)])
        m = {
            "qT8": chunked_T(query[b][sel], 0.125),
            "kT": chunked_T(key[b]),
            "vT": chunked_T(value[b]),
            "wq": wq_h, "wk": wk_h, "wv": wv_h, "wo": wo_h,
            "bq8": bq_h, "bk": bk_h, "bv": bv_h, "bo": bo_h,
            "ones1": ones_h, "ident": id_h,
            "maskA": diag_mask if pair == 0 else full_mask,
            "maskB": zero_mask if pair == 0 else diag_mask,
        }
        in_maps.append(m)
    return in_maps


def kernel(query, key, value, Wq, bq, Wk, bk, Wv, bv, Wo, bo):
    from concourse.bass_utils import run_bass_kernel_spmd

    args = [np.asarray(a, dtype=np.float32) for a in
            (query, key, value, Wq, bq, Wk, bk, Wv, bv, Wo, bo)]
    query, key, value, Wq, bq, Wk, bk, Wv, bv, Wo, bo = args

    nc = _build()
    in_maps = _host_prep(*args)
    res = run_bass_kernel_spmd(nc, in_maps, list(range(8)))

    out = np.empty((B, S, D), np.float32)
    for c in range(8):
        b, pair = c // 2, c % 2
        o = res.results[c]["out"]
        for i in range(NQT):
            g = 2 * i + pair
            out[b, P * g:P * (g + 1), :] = o[P * i:P * (i + 1), :]

    # q = S-1 attends to nothing -> reference softmax is uniform over all keys
    for b in range(B):
        vm = value[b].mean(0)
        x = np.concatenate([vm @ Wv[h] + bv[h] for h in range(H)])
        out[b, S - 1, :] = x @ Wo + bo
    return out


# revision 4
# speedup vs baseline: 1.3043x; 1.3043x over previous
"""Multi-head attention (strictly-upper-triangular mask variant) on 8 TRN2 cores.

Reference math (B=4, S=2048, D=512, H=8, A=64):
    q/k/v = per-head projections of query/key/value           [B,H,S,A]
    scores = q @ k^T / sqrt(A), masked where k <= q (lower triangle incl diag
    masked to -1e9 -> softmax attends strictly to FUTURE positions)
    out = concat_heads(softmax(scores) @ v) @ Wo + bo         [B,S,D]

Sharding: 8 cores = 4 batches x 2 interleaved q-tile sets.  Core c handles
batch b=c//2, q-tiles g = 2*i + (c%2) for i in 0..7 (128 rows each).  Every
core computes all 8 heads for its 1024 query rows; no collectives needed --
the host gather is a pure row-interleave concat.

v2.1 structure (PE was the bottleneck; see _transcript for evolution):
  - inputs DMA'd in first-use order and in chunks so the first projection
    matmuls start as soon as wq + the first qT8 chunk land;
  - PSUM pools: scores scp 2x[128,1024] fp32 (one tile per head per bin,
    double-buffered so ACT exp streams back to back), AV avp 2x[128,512],
    projections/output pjp 2x[128,512];
  - score strips are packed into 9 exactly-1024-wide bins per head; the two
    heads of a pair issue their score matmuls interleaved on disjoint PE row
    groups (partitions 0-63 / 64-127 -> concurrent execution);
  - post-exp masking is a [128,128] elementwise multiply per strip done on
    the otherwise-idle GpSimd engine (pt tiles live in SBUF), keeping both
    the PE and the DVE out of the score->AV chain;
  - softmax is software-pipelined across head pairs: while ACT exponentiates
    pair p's bins, the PE runs AV for pair p-1's second head, AV for pair
    p's first head, and the projections of pair p+1;
  - output projection bias is added by the DVE during PSUM eviction against
    a pre-broadcast bo row block (no K=1 matmuls).
"""

import numpy as np
import ml_dtypes

B, S, D, H, A = 4, 2048, 512, 8, 64
P = 128
NQ = 1024          # q rows per core
NQT = 8            # q tiles per core
NKC = 16           # k chunks
NPAIR = 4          # head pairs
BF = ml_dtypes.bfloat16

# strip widths for the transposed-score layout (strip kc holds the q-tiles
# that need k-chunk kc; width 128*(kc//2+1))
WKC = [P * (kc // 2 + 1) for kc in range(NKC)]

# 9 bins of exactly 1024 columns each
BINS = [(14,), (15,), (12, 0), (13, 1), (10, 2), (11, 3), (8, 4), (9, 5), (6, 7)]
NBIN = len(BINS)
STRIP_BIN = {}   # kc -> (bin index, offset within bin)
for bi, strips in enumerate(BINS):
    off = 0
    for kc in strips:
        STRIP_BIN[kc] = (bi, off)
        off += WKC[kc]
    assert off == 1024

_cache = {}


def _split512(a, b):
    """Split [a,b) at multiples of 512 (PSUM bank boundaries)."""
    out = []
    while a < b:
        nxt = min(b, (a // 512 + 1) * 512)
        out.append((a, nxt))
        a = nxt
    return out


def _build():
    if "nc" in _cache:
        return _cache["nc"]

    import concourse.bacc as bacc
    import concourse.mybir as mybir
    import concourse.tile as tile

    F32 = mybir.dt.float32
    BF16 = mybir.dt.bfloat16
    MULT = mybir.AluOpType.mult
    ADD = mybir.AluOpType.add
    EXP = mybir.ActivationFunctionType.Exp

    nc = bacc.Bacc("TRN2", target_bir_lowering=False, debug=False, num_devices=8)

    qT8_d = nc.dram_tensor("qT8", [P, 4 * NQ], BF16, kind="ExternalInput")
    kT_d = nc.dram_tensor("kT", [P, 4 * S], BF16, kind="ExternalInput")
    vT_d = nc.dram_tensor("vT", [P, 4 * S], BF16, kind="ExternalInput")
    wq_d = nc.dram_tensor("wq", [P, 2048], BF16, kind="ExternalInput")
    wk_d = nc.dram_tensor("wk", [P, 2048], BF16, kind="ExternalInput")
    wv_d = nc.dram_tensor("wv", [P, 2048], BF16, kind="ExternalInput")
    wo_d = nc.dram_tensor("wo", [P, 2048], BF16, kind="ExternalInput")
    bq_d = nc.dram_tensor("bq8", [P, 4], F32, kind="ExternalInput")
    bk_d = nc.dram_tensor("bk", [P, 4], F32, kind="ExternalInput")
    bv_d = nc.dram_tensor("bv", [P, 4], F32, kind="ExternalInput")
    bo_d = nc.dram_tensor("bo128", [P, D], BF16, kind="ExternalInput")
    id_d = nc.dram_tensor("ident", [P, P], BF16, kind="ExternalInput")
    me_d = nc.dram_tensor("maskE", [P, P], BF16, kind="ExternalInput")
    mo_d = nc.dram_tensor("maskO", [P, P], BF16, kind="ExternalInput")
    out_d = nc.dram_tensor("out", [NQ, D], F32, kind="ExternalOutput")

    with tile.TileContext(nc) as tc:
        with (
            tc.tile_pool(name="cst", bufs=1) as cst,
            tc.tile_pool(name="act", bufs=1) as act,
            tc.tile_pool(name="vtt", bufs=2) as vtt,
            tc.tile_pool(name="ptp", bufs=28) as ptp,
            tc.tile_pool(name="rcp", bufs=3) as rcp,
            tc.tile_pool(name="ost", bufs=2) as ost,
            tc.tile_pool(name="scp", bufs=2, space="PSUM") as scp,
            tc.tile_pool(name="avp", bufs=2, space="PSUM") as avp,
            tc.tile_pool(name="pjp", bufs=2, space="PSUM") as pjp,
        ):
            # ---- SBUF homes for the inputs ----
            qT8 = cst.tile([P, 4 * NQ], BF16, tag="qT8")
            kT = cst.tile([P, 4 * S], BF16, tag="kT")
            vT = cst.tile([P, 4 * S], BF16, tag="vT")
            wq = cst.tile([P, 2048], BF16, tag="wq")
            wk = cst.tile([P, 2048], BF16, tag="wk")
            wv = cst.tile([P, 2048], BF16, tag="wv")
            wo = cst.tile([P, 2048], BF16, tag="wo")
            bq = cst.tile([P, 4], F32, tag="bq")
            bk = cst.tile([P, 4], F32, tag="bk")
            bv = cst.tile([P, 4], F32, tag="bv")
            bo = cst.tile([P, D], BF16, tag="bo")
            idt = cst.tile([P, P], BF16, tag="idt")
            mE = cst.tile([P, P], BF16, tag="mE")
            mO = cst.tile([P, P], BF16, tag="mO")
            warm = cst.tile([1, 2], BF16, tag="warm")

            # DMAs in first-use order
            for t, d in [(idt, id_d), (mE, me_d), (mO, mo_d), (bq, bq_d),
                         (bk, bk_d), (bv, bv_d)]:
                nc.sync.dma_start(t[:], d[:])
            # early dummy exp: forces the ACT table load off the critical path
            nc.scalar.activation(warm[:], idt[0:1, 0:2], EXP)
            nc.sync.dma_start(wq[:], wq_d[:])
            for ch in range(4):
                nc.sync.dma_start(qT8[:, NQ * ch:NQ * (ch + 1)],
                                  qT8_d[:, NQ * ch:NQ * (ch + 1)])
            nc.sync.dma_start(wk[:], wk_d[:])
            for ch in range(4):
                nc.sync.dma_start(kT[:, S * ch:S * (ch + 1)],
                                  kT_d[:, S * ch:S * (ch + 1)])
            nc.sync.dma_start(wv[:], wv_d[:])
            for ch in range(4):
                nc.sync.dma_start(vT[:, S * ch:S * (ch + 1)],
                                  vT_d[:, S * ch:S * (ch + 1)])
            nc.sync.dma_start(wo[:], wo_d[:])
            nc.sync.dma_start(bo[:], bo_d[:])

            QT = [act.tile([P, NQ], BF16, tag=f"QT{p}", name=f"QT{p}") for p in range(NPAIR)]
            KT = [act.tile([P, S], BF16, tag=f"KT{p}", name=f"KT{p}") for p in range(NPAIR)]
            Vn = [act.tile([P, NKC * 192], BF16, tag=f"Vn{p}", name=f"Vn{p}") for p in range(NPAIR)]
            XT = [act.tile([P, NQ], BF16, tag=f"XT{c}", name=f"XT{c}") for c in range(4)]
            pts = [[[None] * NBIN for _ in range(2)] for _ in range(NPAIR)]

            # ---- projections for one head pair ----
            def project(p):
                # Q: 2 PSUM tiles of 512
                for qh in range(2):
                    ps = pjp.tile([P, 512], F32, tag="pj")
                    for ch in range(4):
                        nc.tensor.matmul(
                            ps[:],
                            wq[:, (4 * p + ch) * P:(4 * p + ch + 1) * P],
                            qT8[:, NQ * ch + 512 * qh: NQ * ch + 512 * (qh + 1)],
                            start=(ch == 0), stop=(ch == 3))
                    nc.vector.tensor_scalar_add(
                        QT[p][:, 512 * qh:512 * (qh + 1)], ps[:], bq[:, p:p + 1])
                for sh in range(4):
                    ps = pjp.tile([P, 512], F32, tag="pj")
                    for ch in range(4):
                        nc.tensor.matmul(
                            ps[:],
                            wk[:, (4 * p + ch) * P:(4 * p + ch + 1) * P],
                            kT[:, S * ch + 512 * sh: S * ch + 512 * (sh + 1)],
                            start=(ch == 0), stop=(ch == 3))
                    nc.vector.tensor_scalar_add(
                        KT[p][:, 512 * sh:512 * (sh + 1)], ps[:], bk[:, p:p + 1])
                VTp = vtt.tile([P, S], BF16, tag="VT")
                for sh in range(4):
                    ps = pjp.tile([P, 512], F32, tag="pj")
                    for ch in range(4):
                        nc.tensor.matmul(
                            ps[:],
                            wv[:, (4 * p + ch) * P:(4 * p + ch + 1) * P],
                            vT[:, S * ch + 512 * sh: S * ch + 512 * (sh + 1)],
                            start=(ch == 0), stop=(ch == 3))
                    nc.vector.tensor_scalar_add(
                        VTp[:, 512 * sh:512 * (sh + 1)], ps[:], bv[:, p:p + 1])
                # V natural layout blocks [V_h0 | ones | V_h1] per k-chunk
                nc.gpsimd.memset(Vn[p][:], 1.0)
                for m in range(4):
                    vps = pjp.tile([P, 512], BF16, tag="pj",
                                   padded_shape=[P, 1024])
                    for j in range(4):
                        nc.tensor.transpose(
                            vps[:, P * j:P * (j + 1)],
                            VTp[:, P * (4 * m + j):P * (4 * m + j + 1)], idt[:])
                    src = vps[:].rearrange("p (j c) -> p j c", c=P)
                    dst = Vn[p][:].rearrange("p (k c) -> p k c", c=192)
                    nc.vector.tensor_copy(dst[:, 4 * m:4 * m + 4, 0:64],
                                          src[:, :, 0:64])
                    nc.vector.tensor_copy(dst[:, 4 * m:4 * m + 4, 128:192],
                                          src[:, :, 64:128])

            # ---- scores + exp + mask for one pair, one bin (both heads) ----
            # Score matmuls for the two heads use disjoint PE row groups
            # (partitions 0-63 / 64-127) and are emitted interleaved so the
            # hardware runs them concurrently.  Masks multiply the strip's
            # last 128 columns post-exp on GpSimd (pt is SBUF-resident).
            def score_bin(p, bi):
                sc = [scp.tile([P, 1024], F32, tag="sc", name=f"sc{p}_{bi}_{hh}")
                      for hh in range(2)]
                subs = []
                for kc in BINS[bi]:
                    _, F = STRIP_BIN[kc]
                    for (a0, a1) in _split512(F, F + WKC[kc]):
                        subs.append((kc, F, a0, a1))
                for (kc, F, a0, a1) in subs:
                    for hh in range(2):
                        hr = slice(64 * hh, 64 * hh + 64)
                        nc.tensor.matmul(
                            sc[hh][:, a0:a1],
                            KT[p][hr, P * kc:P * (kc + 1)],
                            QT[p][hr, a0 - F:a1 - F],
                            start=True, stop=True)
                for hh in range(2):
                    pt = ptp.tile([P, 1024], BF16, tag="pt",
                                  name=f"pt{p}_{bi}_{hh}")
                    nc.scalar.activation(pt[:], sc[hh][:], EXP)
                    for kc in BINS[bi]:
                        _, F = STRIP_BIN[kc]
                        W = WKC[kc]
                        nc.gpsimd.tensor_tensor(
                            pt[:, F + W - P:F + W], pt[:, F + W - P:F + W],
                            mE[:] if kc % 2 == 0 else mO[:], MULT)
                    pts[p][hh][bi] = pt

            # ---- AV + normalize for one head ----
            def attention_av(h):
                p, hh = h // 2, h % 2
                hr = slice(64 * hh, 64 * hh + 64)
                orow = 0 if hh == 0 else 64
                drow = 64 - orow
                for b in range(2):
                    avb = avp.tile([P, 512], F32, tag="av")
                    kcs = list(range(8 * b, NKC))
                    for n, kc in enumerate(kcs):
                        bi, F = STRIP_BIN[kc]
                        w = min(WKC[kc], 512 * (b + 1)) - 512 * b
                        nc.tensor.matmul(
                            avb[:, 0:w],
                            Vn[p][:, 192 * kc + 64 * hh: 192 * kc + 64 * hh + 128],
                            pts[p][hh][bi][:, F + 512 * b: F + 512 * b + w],
                            start=(n == 0), stop=(n == len(kcs) - 1),
                            skip_group_check=True)
                    # approx-fast reciprocal cannot read PSUM; bounce the
                    # replicated denominators through SBUF
                    rec = rcp.tile([64, 1024], F32, tag="rec")
                    nc.vector.tensor_copy(rec[:, 0:512], avb[drow:drow + 64, :])
                    nc.vector.reciprocal_approx_fast(rec[:, 512:1024], rec[:, 0:512])
                    nc.vector.tensor_tensor(XT[p][hr, 512 * b:512 * (b + 1)],
                                            avb[orow:orow + 64, :], rec[:, 512:1024],
                                            MULT)

            # ---- pipelined emission ----
            project(0)
            for p in range(NPAIR):
                for bi in range(NBIN):
                    score_bin(p, bi)
                if p > 0:
                    attention_av(2 * (p - 1) + 1)   # h1 of previous pair
                attention_av(2 * p)                 # h0 of this pair
                if p < NPAIR - 1:
                    project(p + 1)
            attention_av(2 * (NPAIR - 1) + 1)

            # ---- output projection ----
            for i in range(NQT):
                po = pjp.tile([P, 512], F32, tag="pj")
                for ch in range(4):
                    nc.tensor.matmul(po[:], XT[ch][:, P * i:P * (i + 1)],
                                     wo[:, 512 * ch:512 * (ch + 1)],
                                     start=(ch == 0), stop=(ch == 3))
                ob = ost.tile([P, D], F32, tag="ob")
                nc.vector.tensor_tensor(ob[:], po[:], bo[:], ADD)
                nc.sync.dma_start(out_d[P * i:P * (i + 1), :], ob[:])

    nc.compile()
    _cache["nc"] = nc
    return nc


def _host_prep(query, key, value, Wq, bq, Wk, bk, Wv, bv, Wo, bo):
    """Build the 8 per-core input maps (all device-side layouts)."""
    def stack_pairs(W):
        # [H,D,A] -> [128, 16*128]: col block (4p+ch) = rows 128ch of [Wq_2p|Wq_2p+1]
        blocks = []
        for p in range(NPAIR):
            Wp = np.concatenate([W[2 * p], W[2 * p + 1]], axis=1)  # [512, 128]
            for ch in range(4):
                blocks.append(Wp[P * ch:P * (ch + 1), :])
        return np.stack(blocks, 1).reshape(P, -1).astype(BF)  # [128,2048]

    wq_h, wk_h, wv_h = stack_pairs(Wq), stack_pairs(Wk), stack_pairs(Wv)
    wo_h = np.stack([Wo[P * ch:P * (ch + 1), :] for ch in range(4)], 1)
    wo_h = wo_h.reshape(P, -1).astype(BF)  # [128, 4*512]

    def stack_bias(b, scale=1.0):
        cols = [np.concatenate([b[2 * p], b[2 * p + 1]]) * scale for p in range(NPAIR)]
        return np.stack(cols, 1).astype(np.float32)  # [128, 4]

    bq_h = stack_bias(bq, 0.125)
    bk_h, bv_h = stack_bias(bk), stack_bias(bv)
    bo_h = np.broadcast_to(bo[None, :], (P, D)).astype(BF)
    id_h = np.eye(P, dtype=BF)
    kl = np.arange(P)[:, None]
    ql = np.arange(P)[None, :]
    tril_strict = (kl > ql).astype(BF)

    def chunked_T(x, scale=1.0):
        # [S', D] -> [128, 4*S'] with col block ch = rows 128ch of x.T
        xT = np.ascontiguousarray(x.T) * scale  # [512, S']
        return xT.reshape(4, P, -1).transpose(1, 0, 2).reshape(P, -1).astype(BF)

    in_maps = []
    for c in range(8):
        b, pair = c // 2, c % 2
        sel = np.concatenate(
            [np.arange(P * (2 * i + pair), P * (2 * i + pair) + P) for i in range(NQT)])
        m = {
            "qT8": chunked_T(query[b][sel], 0.125),
            "kT": chunked_T(key[b]),
            "vT": chunked_T(value[b]),
            "wq": wq_h, "wk": wk_h, "wv": wv_h, "wo": wo_h,
            "bq8": bq_h, "bk": bk_h, "bv": bv_h, "bo128": bo_h,
            "ident": id_h,
            "maskE": tril_strict if pair == 0 else np.zeros((P, P), BF),
            "maskO": np.ones((P, P), BF) if pair == 0 else tril_strict,
        }
        in_maps.append(m)
    return in_maps


def kernel(query, key, value, Wq, bq, Wk, bk, Wv, bv, Wo, bo):
    from concourse.bass_utils import run_bass_kernel_spmd

    args = [np.asarray(a, dtype=np.float32) for a in
            (query, key, value, Wq, bq, Wk, bk, Wv, bv, Wo, bo)]
    query, key, value, Wq, bq, Wk, bk, Wv, bv, Wo, bo = args

    nc = _build()
    in_maps = _host_prep(*args)
    res = run_bass_kernel_spmd(nc, in_maps, list(range(8)))

    out = np.empty((B, S, D), np.float32)
    for c in range(8):
        b, pair = c // 2, c % 2
        o = res.results[c]["out"]
        for i in range(NQT):
            g = 2 * i + pair
            out[b, P * g:P * (g + 1), :] = o[P * i:P * (i + 1), :]

    # q = S-1 attends to nothing -> reference softmax is uniform over all keys
    for b in range(B):
        vm = value[b].mean(0)
        x = np.concatenate([vm @ Wv[h] + bv[h] for h in range(H)])
        out[b, S - 1, :] = x @ Wo + bo
    return out


# revision 9
# speedup vs baseline: 1.3552x; 1.0390x over previous
"""Multi-head attention (strictly-upper-triangular mask variant) on 8 TRN2 cores.

Reference math (B=4, S=2048, D=512, H=8, A=64):
    q/k/v = per-head projections of query/key/value           [B,H,S,A]
    scores = q @ k^T / sqrt(A), masked where k <= q (lower triangle incl diag
    masked to -1e9 -> softmax attends strictly to FUTURE positions)
    out = concat_heads(softmax(scores) @ v) @ Wo + bo         [B,S,D]

Sharding: 8 cores = 4 batches x 2 interleaved q-tile sets.  Core c handles
batch b=c//2, q-tiles g = 2*i + (c%2) for i in 0..7 (128 rows each).  Every
core computes all 8 heads for its 1024 query rows; no collectives needed --
the host gather is a pure row-interleave concat.

v2.1 structure (PE was the bottleneck; see _transcript for evolution):
  - inputs DMA'd in first-use order and in chunks so the first projection
    matmuls start as soon as wq + the first qT8 chunk land;
  - PSUM pools: scores scp 2x[128,1024] fp32 (one tile per head per bin,
    double-buffered so ACT exp streams back to back), AV avp 2x[128,512],
    projections/output pjp 2x[128,512];
  - score strips are packed into 9 exactly-1024-wide bins per head; the two
    heads of a pair issue their score matmuls interleaved on disjoint PE row
    groups (partitions 0-63 / 64-127 -> concurrent execution);
  - post-exp masking is a [128,128] elementwise multiply per strip done on
    the otherwise-idle GpSimd engine (pt tiles live in SBUF), keeping both
    the PE and the DVE out of the score->AV chain;
  - softmax is software-pipelined across head pairs: while ACT exponentiates
    pair p's bins, the PE runs AV for pair p-1's second head, AV for pair
    p's first head, and the projections of pair p+1;
  - output projection bias is added by the DVE during PSUM eviction against
    a pre-broadcast bo row block (no K=1 matmuls).
"""

import numpy as np
import ml_dtypes

B, S, D, H, A = 4, 2048, 512, 8, 64
P = 128
NQ = 1024          # q rows per core
NQT = 8            # q tiles per core
NKC = 16           # k chunks
NPAIR = 4          # head pairs
BF = ml_dtypes.bfloat16

# strip widths for the transposed-score layout (strip kc holds the q-tiles
# that need k-chunk kc; width 128*(kc//2+1))
WKC = [P * (kc // 2 + 1) for kc in range(NKC)]

# 9 bins of exactly 1024 columns each
BINS = [(14,), (15,), (12, 0), (13, 1), (10, 2), (11, 3), (8, 4), (9, 5), (6, 7)]
NBIN = len(BINS)
STRIP_BIN = {}   # kc -> (bin index, offset within bin)
for bi, strips in enumerate(BINS):
    off = 0
    for kc in strips:
        STRIP_BIN[kc] = (bi, off)
        off += WKC[kc]
    assert off == 1024

_cache = {}


def _split512(a, b):
    """Split [a,b) at multiples of 512 (PSUM bank boundaries)."""
    out = []
    while a < b:
        nxt = min(b, (a // 512 + 1) * 512)
        out.append((a, nxt))
        a = nxt
    return out


def _build():
    if "nc" in _cache:
        return _cache["nc"]

    import concourse.bacc as bacc
    import concourse.mybir as mybir
    import concourse.tile as tile

    F32 = mybir.dt.float32
    BF16 = mybir.dt.bfloat16
    MULT = mybir.AluOpType.mult
    ADD = mybir.AluOpType.add
    EXP = mybir.ActivationFunctionType.Exp

    nc = bacc.Bacc("TRN2", target_bir_lowering=False, debug=False, num_devices=8)

    qT8_d = nc.dram_tensor("qT8", [P, 4 * NQ], BF16, kind="ExternalInput")
    kT_d = nc.dram_tensor("kT", [P, 4 * S], BF16, kind="ExternalInput")
    vT_d = nc.dram_tensor("vT", [P, 4 * S], BF16, kind="ExternalInput")
    wq_d = nc.dram_tensor("wq", [P, 2048], BF16, kind="ExternalInput")
    wk_d = nc.dram_tensor("wk", [P, 2048], BF16, kind="ExternalInput")
    wv_d = nc.dram_tensor("wv", [P, 2048], BF16, kind="ExternalInput")
    wo_d = nc.dram_tensor("wo", [P, 2048], BF16, kind="ExternalInput")
    bq_d = nc.dram_tensor("bq8", [P, 4], F32, kind="ExternalInput")
    bk_d = nc.dram_tensor("bk", [P, 4], F32, kind="ExternalInput")
    bv_d = nc.dram_tensor("bv", [P, 4], F32, kind="ExternalInput")
    bo_d = nc.dram_tensor("bo128", [P, D], BF16, kind="ExternalInput")
    id_d = nc.dram_tensor("ident", [P, P], BF16, kind="ExternalInput")
    me_d = nc.dram_tensor("maskE", [P, P], BF16, kind="ExternalInput")
    mo_d = nc.dram_tensor("maskO", [P, P], BF16, kind="ExternalInput")
    out_d = nc.dram_tensor("out", [NQ, D], F32, kind="ExternalOutput")

    with tile.TileContext(nc) as tc:
        with (
            tc.tile_pool(name="cst", bufs=1) as cst,
            tc.tile_pool(name="act", bufs=1) as act,
            tc.tile_pool(name="vtt", bufs=2) as vtt,
            tc.tile_pool(name="ptp", bufs=28) as ptp,
            tc.tile_pool(name="rcp", bufs=3) as rcp,
            tc.tile_pool(name="ost", bufs=2) as ost,
            tc.tile_pool(name="scp", bufs=2, space="PSUM") as scp,
            tc.tile_pool(name="avp", bufs=2, space="PSUM") as avp,
            tc.tile_pool(name="pjp", bufs=2, space="PSUM") as pjp,
        ):
            # ---- SBUF homes for the inputs ----
            qT8 = cst.tile([P, 4 * NQ], BF16, tag="qT8")
            kT = cst.tile([P, 4 * S], BF16, tag="kT")
            vT = cst.tile([P, 4 * S], BF16, tag="vT")
            wq = cst.tile([P, 2048], BF16, tag="wq")
            wk = cst.tile([P, 2048], BF16, tag="wk")
            wv = cst.tile([P, 2048], BF16, tag="wv")
            wo = cst.tile([P, 2048], BF16, tag="wo")
            bq = cst.tile([P, 4], F32, tag="bq")
            bk = cst.tile([P, 4], F32, tag="bk")
            bv = cst.tile([P, 4], F32, tag="bv")
            bo = cst.tile([P, D], BF16, tag="bo")
            idt = cst.tile([P, P], BF16, tag="idt")
            mE = cst.tile([P, P], BF16, tag="mE")
            mO = cst.tile([P, P], BF16, tag="mO")
            warm = cst.tile([1, 2], BF16, tag="warm")

            # DMAs in first-use order.  Weight layouts are ch-major (column
            # block 4*ch+p) so a per-ch chunk is contiguous and the first
            # projection matmul only waits for chunk 0.
            for t, d in [(idt, id_d), (mE, me_d), (mO, mo_d), (bq, bq_d),
                         (bk, bk_d), (bv, bv_d)]:
                nc.sync.dma_start(t[:], d[:])
            # early dummy exp: forces the ACT table load off the critical path
            nc.scalar.activation(warm[:], idt[0:1, 0:2], EXP)
            for ch in range(4):
                nc.sync.dma_start(wq[:, 512 * ch:512 * (ch + 1)],
                                  wq_d[:, 512 * ch:512 * (ch + 1)])
                nc.sync.dma_start(qT8[:, NQ * ch:NQ * (ch + 1)],
                                  qT8_d[:, NQ * ch:NQ * (ch + 1)])
            for ch in range(4):
                nc.sync.dma_start(wk[:, 512 * ch:512 * (ch + 1)],
                                  wk_d[:, 512 * ch:512 * (ch + 1)])
                nc.sync.dma_start(kT[:, S * ch:S * (ch + 1)],
                                  kT_d[:, S * ch:S * (ch + 1)])
            for ch in range(4):
                nc.sync.dma_start(wv[:, 512 * ch:512 * (ch + 1)],
                                  wv_d[:, 512 * ch:512 * (ch + 1)])
                nc.sync.dma_start(vT[:, S * ch:S * (ch + 1)],
                                  vT_d[:, S * ch:S * (ch + 1)])
            nc.sync.dma_start(wo[:], wo_d[:])
            nc.sync.dma_start(bo[:], bo_d[:])

            QT = [act.tile([P, NQ], BF16, tag=f"QT{p}", name=f"QT{p}") for p in range(NPAIR)]
            KT = [act.tile([P, S], BF16, tag=f"KT{p}", name=f"KT{p}") for p in range(NPAIR)]
            Vn = [act.tile([P, NKC * 192], BF16, tag=f"Vn{p}", name=f"Vn{p}") for p in range(NPAIR)]
            XT = [act.tile([P, NQ], BF16, tag=f"XT{c}", name=f"XT{c}") for c in range(4)]
            pts = [[[None] * NBIN for _ in range(2)] for _ in range(NPAIR)]

            # ---- projections for one head pair ----
            # K chunks are projected in bin-consumption order (sh=3 holds
            # k-chunks 12-15 which feed score bins 0-3) so scores unblock
            # after the first K eviction.
            def project(p):
                # Q: 2 PSUM tiles of 512
                for qh in range(2):
                    ps = pjp.tile([P, 512], F32, tag="pj")
                    for ch in range(4):
                        nc.tensor.matmul(
                            ps[:],
                            wq[:, 512 * ch + P * p:512 * ch + P * (p + 1)],
                            qT8[:, NQ * ch + 512 * qh: NQ * ch + 512 * (qh + 1)],
                            start=(ch == 0), stop=(ch == 3))
                    nc.vector.tensor_scalar_add(
                        QT[p][:, 512 * qh:512 * (qh + 1)], ps[:], bq[:, p:p + 1])
                for sh in (3, 0, 2, 1):
                    ps = pjp.tile([P, 512], F32, tag="pj")
                    for ch in range(4):
                        nc.tensor.matmul(
                            ps[:],
                            wk[:, 512 * ch + P * p:512 * ch + P * (p + 1)],
                            kT[:, S * ch + 512 * sh: S * ch + 512 * (sh + 1)],
                            start=(ch == 0), stop=(ch == 3))
                    nc.vector.tensor_scalar_add(
                        KT[p][:, 512 * sh:512 * (sh + 1)], ps[:], bk[:, p:p + 1])
                VTp = vtt.tile([P, S], BF16, tag="VT")
                for sh in range(4):
                    ps = pjp.tile([P, 512], F32, tag="pj")
                    for ch in range(4):
                        nc.tensor.matmul(
                            ps[:],
                            wv[:, 512 * ch + P * p:512 * ch + P * (p + 1)],
                            vT[:, S * ch + 512 * sh: S * ch + 512 * (sh + 1)],
                            start=(ch == 0), stop=(ch == 3))
                    nc.vector.tensor_scalar_add(
                        VTp[:, 512 * sh:512 * (sh + 1)], ps[:], bv[:, p:p + 1])
                # V natural layout blocks [V_h0 | ones | V_h1] per k-chunk
                nc.gpsimd.memset(Vn[p][:], 1.0)
                for m in range(4):
                    vps = pjp.tile([P, 512], BF16, tag="pj",
                                   padded_shape=[P, 1024])
                    for j in range(4):
                        nc.tensor.transpose(
                            vps[:, P * j:P * (j + 1)],
                            VTp[:, P * (4 * m + j):P * (4 * m + j + 1)], idt[:])
                    src = vps[:].rearrange("p (j c) -> p j c", c=P)
                    dst = Vn[p][:].rearrange("p (k c) -> p k c", c=192)
                    nc.vector.tensor_copy(dst[:, 4 * m:4 * m + 4, 0:64],
                                          src[:, :, 0:64])
                    nc.vector.tensor_copy(dst[:, 4 * m:4 * m + 4, 128:192],
                                          src[:, :, 64:128])

            # ---- scores + exp + mask for one pair, one bin (both heads) ----
            # Score matmuls for the two heads use disjoint PE row groups
            # (partitions 0-63 / 64-127) and are emitted interleaved so the
            # hardware runs them concurrently.  Masks multiply the strip's
            # last 128 columns post-exp on GpSimd (pt is SBUF-resident).
            def score_bin(p, bi):
                sc = [scp.tile([P, 1024], F32, tag="sc", name=f"sc{p}_{bi}_{hh}")
                      for hh in range(2)]
                subs = []
                for kc in BINS[bi]:
                    _, F = STRIP_BIN[kc]
                    for (a0, a1) in _split512(F, F + WKC[kc]):
                        subs.append((kc, F, a0, a1))
                for (kc, F, a0, a1) in subs:
                    for hh in range(2):
                        hr = slice(64 * hh, 64 * hh + 64)
                        nc.tensor.matmul(
                            sc[hh][:, a0:a1],
                            KT[p][hr, P * kc:P * (kc + 1)],
                            QT[p][hr, a0 - F:a1 - F],
                            start=True, stop=True)
                for hh in range(2):
                    pt = ptp.tile([P, 1024], BF16, tag="pt",
                                  name=f"pt{p}_{bi}_{hh}")
                    nc.scalar.activation(pt[:], sc[hh][:], EXP)
                    for kc in BINS[bi]:
                        _, F = STRIP_BIN[kc]
                        W = WKC[kc]
                        nc.gpsimd.tensor_tensor(
                            pt[:, F + W - P:F + W], pt[:, F + W - P:F + W],
                            mE[:] if kc % 2 == 0 else mO[:], MULT)
                    pts[p][hh][bi] = pt

            # ---- AV + normalize for one head ----
            def attention_av(h, banks=(0, 1)):
                p, hh = h // 2, h % 2
                hr = slice(64 * hh, 64 * hh + 64)
                orow = 0 if hh == 0 else 64
                drow = 64 - orow
                for b in banks:
                    avb = avp.tile([P, 512], F32, tag="av")
                    kcs = list(range(8 * b, NKC))
                    for n, kc in enumerate(kcs):
                        bi, F = STRIP_BIN[kc]
                        w = min(WKC[kc], 512 * (b + 1)) - 512 * b
                        nc.tensor.matmul(
                            avb[:, 0:w],
                            Vn[p][:, 192 * kc + 64 * hh: 192 * kc + 64 * hh + 128],
                            pts[p][hh][bi][:, F + 512 * b: F + 512 * b + w],
                            start=(n == 0), stop=(n == len(kcs) - 1),
                            skip_group_check=True)
                    # approx-fast reciprocal cannot read PSUM; bounce the
                    # replicated denominators through SBUF
                    rec = rcp.tile([64, 1024], F32, tag="rec")
                    nc.vector.tensor_copy(rec[:, 0:512], avb[drow:drow + 64, :])
                    nc.vector.reciprocal_approx_fast(rec[:, 512:1024], rec[:, 0:512])
                    nc.vector.tensor_tensor(XT[p][hr, 512 * b:512 * (b + 1)],
                                            avb[orow:orow + 64, :], rec[:, 512:1024],
                                            MULT)

            # ---- output projection for q-tiles [i0, i1) ----
            def out_proj(i0, i1):
                for i in range(i0, i1):
                    po = pjp.tile([P, 512], F32, tag="pj")
                    for ch in range(4):
                        nc.tensor.matmul(po[:], XT[ch][:, P * i:P * (i + 1)],
                                         wo[:, 512 * ch:512 * (ch + 1)],
                                         start=(ch == 0), stop=(ch == 3))
                    ob = ost.tile([P, D], F32, tag="ob")
                    nc.vector.tensor_tensor(ob[:], po[:], bo[:], ADD)
                    nc.sync.dma_start(out_d[P * i:P * (i + 1), :], ob[:])

            # ---- pipelined emission ----
            # Per slot: score bins first (they feed ACT, the pacer), then
            # next pair's projections, then AV work (naturally back-loaded).
            project(0)
            for p in range(NPAIR):
                for bi in range(NBIN):
                    score_bin(p, bi)
                if p < NPAIR - 1:
                    project(p + 1)
                if p > 0:
                    attention_av(2 * (p - 1) + 1)   # h1 of previous pair
                attention_av(2 * p)                 # h0 of this pair
            # drain: h1 of pair 3, with the output projection split so its
            # first half overlaps the second AV bank
            attention_av(7, banks=(0,))
            out_proj(0, 4)
            attention_av(7, banks=(1,))
            out_proj(4, 8)

    nc.compile()
    _cache["nc"] = nc
    return nc


def _host_prep(query, key, value, Wq, bq, Wk, bk, Wv, bv, Wo, bo):
    """Build the 8 per-core input maps (all device-side layouts)."""
    def stack_pairs(W):
        # [H,D,A] -> [128, 16*128]: col block (4ch+p) = rows 128ch of
        # [W_2p|W_2p+1] (ch-major so a per-ch DMA chunk is contiguous)
        blocks = []
        for ch in range(4):
            for p in range(NPAIR):
                Wp = np.concatenate([W[2 * p], W[2 * p + 1]], axis=1)  # [512, 128]
                blocks.append(Wp[P * ch:P * (ch + 1), :])
        return np.stack(blocks, 1).reshape(P, -1).astype(BF)  # [128,2048]

    wq_h, wk_h, wv_h = stack_pairs(Wq), stack_pairs(Wk), stack_pairs(Wv)
    wo_h = np.stack([Wo[P * ch:P * (ch + 1), :] for ch in range(4)], 1)
    wo_h = wo_h.reshape(P, -1).astype(BF)  # [128, 4*512]

    def stack_bias(b, scale=1.0):
        cols = [np.concatenate([b[2 * p], b[2 * p + 1]]) * scale for p in range(NPAIR)]
        return np.stack(cols, 1).astype(np.float32)  # [128, 4]

    bq_h = stack_bias(bq, 0.125)
    bk_h, bv_h = stack_bias(bk), stack_bias(bv)
    bo_h = np.broadcast_to(bo[None, :], (P, D)).astype(BF)
    id_h = np.eye(P, dtype=BF)
    kl = np.arange(P)[:, None]
    ql = np.arange(P)[None, :]
    tril_strict = (kl > ql).astype(BF)

    def chunked_T(x, scale=1.0):
        # [S', D] -> [128, 4*S'] with col block ch = rows 128ch of x.T
        xT = np.ascontiguousarray(x.T) * scale  # [512, S']
        return xT.reshape(4, P, -1).transpose(1, 0, 2).reshape(P, -1).astype(BF)

    in_maps = []
    for c in range(8):
        b, pair = c // 2, c % 2
        sel = np.concatenate(
            [np.arange(P * (2 * i + pair), P * (2 * i + pair) + P) for i in range(NQT)])
        m = {
            "qT8": chunked_T(query[b][sel], 0.125),
            "kT": chunked_T(key[b]),
            "vT": chunked_T(value[b]),
            "wq": wq_h, "wk": wk_h, "wv": wv_h, "wo": wo_h,
            "bq8": bq_h, "bk": bk_h, "bv": bv_h, "bo128": bo_h,
            "ident": id_h,
            "maskE": tril_strict if pair == 0 else np.zeros((P, P), BF),
            "maskO": np.ones((P, P), BF) if pair == 0 else tril_strict,
        }
        in_maps.append(m)
    return in_maps


def kernel(query, key, value, Wq, bq, Wk, bk, Wv, bv, Wo, bo):
    from concourse.bass_utils import run_bass_kernel_spmd

    args = [np.asarray(a, dtype=np.float32) for a in
            (query, key, value, Wq, bq, Wk, bk, Wv, bv, Wo, bo)]
    query, key, value, Wq, bq, Wk, bk, Wv, bv, Wo, bo = args

    nc = _build()
    in_maps = _host_prep(*args)
    res = run_bass_kernel_spmd(nc, in_maps, list(range(8)))

    out = np.empty((B, S, D), np.float32)
    for c in range(8):
        b, pair = c // 2, c % 2
        o = res.results[c]["out"]
        for i in range(NQT):
            g = 2 * i + pair
            out[b, P * g:P * (g + 1), :] = o[P * i:P * (i + 1), :]

    # q = S-1 attends to nothing -> reference softmax is uniform over all keys
    for b in range(B):
        vm = value[b].mean(0)
        x = np.concatenate([vm @ Wv[h] + bv[h] for h in range(H)])
        out[b, S - 1, :] = x @ Wo + bo
    return out


# revision 13
# speedup vs baseline: 1.4614x; 1.0784x over previous
"""Multi-head attention (strictly-upper-triangular mask variant) on 8 TRN2 cores.

Reference math (B=4, S=2048, D=512, H=8, A=64):
    q/k/v = per-head projections of query/key/value           [B,H,S,A]
    scores = q @ k^T / sqrt(A), masked where k <= q (lower triangle incl diag
    masked to -1e9 -> softmax attends strictly to FUTURE positions)
    out = concat_heads(softmax(scores) @ v) @ Wo + bo         [B,S,D]

Sharding: 8 cores = 4 batches x 2 interleaved q-tile sets.  Core c handles
batch b=c//2, q-tiles g = 2*i + (c%2) for i in 0..7 (128 rows each).  Every
core computes all 8 heads for its 1024 query rows; no collectives needed --
the host gather is a pure row-interleave concat.

v2.1 structure (PE was the bottleneck; see _transcript for evolution):
  - inputs DMA'd in first-use order and in chunks so the first projection
    matmuls start as soon as wq + the first qT8 chunk land;
  - PSUM pools: scores scp 2x[128,1024] fp32 (one tile per head per bin,
    double-buffered so ACT exp streams back to back), AV avp 2x[128,512],
    projections/output pjp 2x[128,512];
  - score strips are packed into 9 exactly-1024-wide bins per head; the two
    heads of a pair issue their score matmuls interleaved on disjoint PE row
    groups (partitions 0-63 / 64-127 -> concurrent execution);
  - post-exp masking is a [128,128] elementwise multiply per strip done on
    the otherwise-idle GpSimd engine (pt tiles live in SBUF), keeping both
    the PE and the DVE out of the score->AV chain;
  - softmax is software-pipelined across head pairs: while ACT exponentiates
    pair p's bins, the PE runs AV for pair p-1's second head, AV for pair
    p's first head, and the projections of pair p+1;
  - output projection bias is added by the DVE during PSUM eviction against
    a pre-broadcast bo row block (no K=1 matmuls).
"""

import numpy as np
import ml_dtypes

B, S, D, H, A = 4, 2048, 512, 8, 64
P = 128
NQ = 1024          # q rows per core
NQT = 8            # q tiles per core
NKC = 16           # k chunks
NPAIR = 4          # head pairs
BF = ml_dtypes.bfloat16

# strip widths for the transposed-score layout (strip kc holds the q-tiles
# that need k-chunk kc; width 128*(kc//2+1))
WKC = [P * (kc // 2 + 1) for kc in range(NKC)]

# 9 bins of exactly 1024 columns each
BINS = [(14,), (15,), (12, 0), (13, 1), (10, 2), (11, 3), (8, 4), (9, 5), (6, 7)]
NBIN = len(BINS)
STRIP_BIN = {}   # kc -> (bin index, offset within bin)
for bi, strips in enumerate(BINS):
    off = 0
    for kc in strips:
        STRIP_BIN[kc] = (bi, off)
        off += WKC[kc]
    assert off == 1024

_cache = {}


def _split512(a, b):
    """Split [a,b) at multiples of 512 (PSUM bank boundaries)."""
    out = []
    while a < b:
        nxt = min(b, (a // 512 + 1) * 512)
        out.append((a, nxt))
        a = nxt
    return out


def _build():
    if "nc" in _cache:
        return _cache["nc"]

    import concourse.bacc as bacc
    import concourse.mybir as mybir
    import concourse.tile as tile

    F32 = mybir.dt.float32
    BF16 = mybir.dt.bfloat16
    MULT = mybir.AluOpType.mult
    ADD = mybir.AluOpType.add
    EXP = mybir.ActivationFunctionType.Exp

    nc = bacc.Bacc("TRN2", target_bir_lowering=False, debug=False, num_devices=8)

    qT8_d = nc.dram_tensor("qT8", [P, 4 * NQ], BF16, kind="ExternalInput")
    kT_d = nc.dram_tensor("kT", [P, 4 * S], BF16, kind="ExternalInput")
    vT_d = nc.dram_tensor("vT", [P, 4 * S], BF16, kind="ExternalInput")
    wq_d = nc.dram_tensor("wq", [P, 2048], BF16, kind="ExternalInput")
    wk_d = nc.dram_tensor("wk", [P, 2048], BF16, kind="ExternalInput")
    wv_d = nc.dram_tensor("wv", [P, 2048], BF16, kind="ExternalInput")
    wo_d = nc.dram_tensor("wo", [P, 2048], BF16, kind="ExternalInput")
    bq_d = nc.dram_tensor("bq8", [P, 4], F32, kind="ExternalInput")
    bk_d = nc.dram_tensor("bk", [P, 4], F32, kind="ExternalInput")
    bv_d = nc.dram_tensor("bv", [P, 4], F32, kind="ExternalInput")
    bo_d = nc.dram_tensor("bo128", [P, D], BF16, kind="ExternalInput")
    id_d = nc.dram_tensor("ident", [P, P], BF16, kind="ExternalInput")
    me_d = nc.dram_tensor("maskE", [P, P], BF16, kind="ExternalInput")
    mo_d = nc.dram_tensor("maskO", [P, P], BF16, kind="ExternalInput")
    out_d = nc.dram_tensor("out", [NQ, D], F32, kind="ExternalOutput")

    with tile.TileContext(nc) as tc:
        with (
            tc.tile_pool(name="cst", bufs=1) as cst,
            tc.tile_pool(name="act", bufs=1) as act,
            tc.tile_pool(name="vtt", bufs=2) as vtt,
            tc.tile_pool(name="ptp", bufs=28) as ptp,
            tc.tile_pool(name="rcp", bufs=3) as rcp,
            tc.tile_pool(name="ost", bufs=2) as ost,
            tc.tile_pool(name="scp", bufs=2, space="PSUM") as scp,
            tc.tile_pool(name="avp", bufs=2, space="PSUM") as avp,
            tc.tile_pool(name="pjp", bufs=2, space="PSUM") as pjp,
        ):
            # ---- SBUF homes for the inputs ----
            qT8 = cst.tile([P, 4 * NQ], BF16, tag="qT8")
            kT = cst.tile([P, 4 * S], BF16, tag="kT")
            vT = cst.tile([P, 4 * S], BF16, tag="vT")
            wq = cst.tile([P, 2048], BF16, tag="wq")
            wk = cst.tile([P, 2048], BF16, tag="wk")
            wv = cst.tile([P, 2048], BF16, tag="wv")
            wo = cst.tile([P, 2048], BF16, tag="wo")
            bq = cst.tile([P, 4], F32, tag="bq")
            bk = cst.tile([P, 4], F32, tag="bk")
            bv = cst.tile([P, 4], F32, tag="bv")
            bo = cst.tile([P, D], BF16, tag="bo")
            idt = cst.tile([P, P], BF16, tag="idt")
            mE = cst.tile([P, P], BF16, tag="mE")
            mO = cst.tile([P, P], BF16, tag="mO")
            warm = cst.tile([1, 2], BF16, tag="warm")

            # DMAs in first-use order.  Weight layouts are ch-major (column
            # block 4*ch+p) so a per-ch chunk is contiguous and the first
            # projection matmul only waits for chunk 0.
            for t, d in [(idt, id_d), (mE, me_d), (mO, mo_d), (bq, bq_d),
                         (bk, bk_d), (bv, bv_d)]:
                nc.sync.dma_start(t[:], d[:])
            # early dummy exp: forces the ACT table load off the critical path
            nc.scalar.activation(warm[:], idt[0:1, 0:2], EXP)
            for ch in range(4):
                nc.sync.dma_start(wq[:, 512 * ch:512 * (ch + 1)],
                                  wq_d[:, 512 * ch:512 * (ch + 1)])
                nc.sync.dma_start(qT8[:, NQ * ch:NQ * (ch + 1)],
                                  qT8_d[:, NQ * ch:NQ * (ch + 1)])
            for ch in range(4):
                nc.sync.dma_start(wk[:, 512 * ch:512 * (ch + 1)],
                                  wk_d[:, 512 * ch:512 * (ch + 1)])
                nc.sync.dma_start(kT[:, S * ch:S * (ch + 1)],
                                  kT_d[:, S * ch:S * (ch + 1)])
            for ch in range(4):
                nc.sync.dma_start(wv[:, 512 * ch:512 * (ch + 1)],
                                  wv_d[:, 512 * ch:512 * (ch + 1)])
                nc.sync.dma_start(vT[:, S * ch:S * (ch + 1)],
                                  vT_d[:, S * ch:S * (ch + 1)])
            nc.sync.dma_start(wo[:], wo_d[:])
            nc.sync.dma_start(bo[:], bo_d[:])

            # Per-head zero-padded Q operands: QTz[p][hh] holds the head's
            # 64 Q rows in its own partition range and ZEROS in the other
            # head's range.  Score matmuls then run with the full [128,128]
            # KT block as stationary (the zeros kill the other head's
            # contribution), so the whole kernel stays in 128-row PE mode --
            # no tiling-mode switches, which drain the PE array.
            QTz = [[act.tile([P, NQ], BF16, tag=f"QTz{p}_{hh}", name=f"QTz{p}_{hh}")
                    for hh in range(2)] for p in range(NPAIR)]
            KT = [act.tile([P, S], BF16, tag=f"KT{p}", name=f"KT{p}") for p in range(NPAIR)]
            Vn = [act.tile([P, NKC * 192], BF16, tag=f"Vn{p}", name=f"Vn{p}") for p in range(NPAIR)]
            XT = [act.tile([P, NQ], BF16, tag=f"XT{c}", name=f"XT{c}") for c in range(4)]
            pts = [[[None] * NBIN for _ in range(2)] for _ in range(NPAIR)]
            for p in range(NPAIR):
                nc.gpsimd.memset(QTz[p][0][64:P, :], 0.0)
                nc.gpsimd.memset(QTz[p][1][0:64, :], 0.0)

            # ---- projections for one head pair ----
            # K chunks are projected in bin-consumption order (sh=3 holds
            # k-chunks 12-15 which feed score bins 0-3) so scores unblock
            # after the first K eviction.
            def project(p):
                # Q: 2 PSUM tiles of 512, evicted per head half into the
                # zero-padded QTz operands
                for qh in range(2):
                    ps = pjp.tile([P, 512], F32, tag="pj")
                    for ch in range(4):
                        nc.tensor.matmul(
                            ps[:],
                            wq[:, 512 * ch + P * p:512 * ch + P * (p + 1)],
                            qT8[:, NQ * ch + 512 * qh: NQ * ch + 512 * (qh + 1)],
                            start=(ch == 0), stop=(ch == 3))
                    for hh in range(2):
                        hr = slice(64 * hh, 64 * hh + 64)
                        nc.vector.tensor_scalar_add(
                            QTz[p][hh][hr, 512 * qh:512 * (qh + 1)],
                            ps[hr, :], bq[64 * hh:64 * hh + 64, p:p + 1])
                for sh in (3, 0, 2, 1):
                    ps = pjp.tile([P, 512], F32, tag="pj")
                    for ch in range(4):
                        nc.tensor.matmul(
                            ps[:],
                            wk[:, 512 * ch + P * p:512 * ch + P * (p + 1)],
                            kT[:, S * ch + 512 * sh: S * ch + 512 * (sh + 1)],
                            start=(ch == 0), stop=(ch == 3))
                    nc.vector.tensor_scalar_add(
                        KT[p][:, 512 * sh:512 * (sh + 1)], ps[:], bk[:, p:p + 1])
                VTp = vtt.tile([P, S], BF16, tag="VT")
                for sh in range(4):
                    ps = pjp.tile([P, 512], F32, tag="pj")
                    for ch in range(4):
                        nc.tensor.matmul(
                            ps[:],
                            wv[:, 512 * ch + P * p:512 * ch + P * (p + 1)],
                            vT[:, S * ch + 512 * sh: S * ch + 512 * (sh + 1)],
                            start=(ch == 0), stop=(ch == 3))
                    nc.vector.tensor_scalar_add(
                        VTp[:, 512 * sh:512 * (sh + 1)], ps[:], bv[:, p:p + 1])
                # V natural layout blocks [V_h0 | ones | V_h1] per k-chunk
                nc.gpsimd.memset(Vn[p][:], 1.0)
                for m in range(4):
                    vps = pjp.tile([P, 512], BF16, tag="pj",
                                   padded_shape=[P, 1024])
                    for j in range(4):
                        nc.tensor.transpose(
                            vps[:, P * j:P * (j + 1)],
                            VTp[:, P * (4 * m + j):P * (4 * m + j + 1)], idt[:])
                    src = vps[:].rearrange("p (j c) -> p j c", c=P)
                    dst = Vn[p][:].rearrange("p (k c) -> p k c", c=192)
                    nc.vector.tensor_copy(dst[:, 4 * m:4 * m + 4, 0:64],
                                          src[:, :, 0:64])
                    nc.vector.tensor_copy(dst[:, 4 * m:4 * m + 4, 128:192],
                                          src[:, :, 64:128])

            # ---- scores + exp + mask for one pair, one bin (both heads) ----
            # Score matmuls for the two heads use disjoint PE row groups
            # (partitions 0-63 / 64-127) and are emitted interleaved so the
            # hardware runs them concurrently.  Masks multiply the strip's
            # last 128 columns post-exp on GpSimd (pt is SBUF-resident).
            def score_bin(p, bi):
                sc = [scp.tile([P, 1024], F32, tag="sc", name=f"sc{p}_{bi}_{hh}")
                      for hh in range(2)]
                subs = []
                for kc in BINS[bi]:
                    _, F = STRIP_BIN[kc]
                    for (a0, a1) in _split512(F, F + WKC[kc]):
                        subs.append((kc, F, a0, a1))
                for (kc, F, a0, a1) in subs:
                    for hh in range(2):
                        nc.tensor.matmul(
                            sc[hh][:, a0:a1],
                            KT[p][:, P * kc:P * (kc + 1)],
                            QTz[p][hh][:, a0 - F:a1 - F],
                            start=True, stop=True)
                for hh in range(2):
                    pt = ptp.tile([P, 1024], BF16, tag="pt",
                                  name=f"pt{p}_{bi}_{hh}")
                    nc.scalar.activation(pt[:], sc[hh][:], EXP)
                    for kc in BINS[bi]:
                        _, F = STRIP_BIN[kc]
                        W = WKC[kc]
                        nc.gpsimd.tensor_tensor(
                            pt[:, F + W - P:F + W], pt[:, F + W - P:F + W],
                            mE[:] if kc % 2 == 0 else mO[:], MULT)
                    pts[p][hh][bi] = pt

            # ---- AV + normalize for one head ----
            def attention_av(h, banks=(0, 1)):
                p, hh = h // 2, h % 2
                hr = slice(64 * hh, 64 * hh + 64)
                orow = 0 if hh == 0 else 64
                drow = 64 - orow
                for b in banks:
                    avb = avp.tile([P, 512], F32, tag="av")
                    # accumulate in bin-completion order so the chain's last
                    # matmul is ready right after the last exp+mask
                    kcs = [kc for bj in range(NBIN) for kc in BINS[bj]
                           if kc >= 8 * b]
                    for n, kc in enumerate(kcs):
                        bi, F = STRIP_BIN[kc]
                        w = min(WKC[kc], 512 * (b + 1)) - 512 * b
                        nc.tensor.matmul(
                            avb[:, 0:w],
                            Vn[p][:, 192 * kc + 64 * hh: 192 * kc + 64 * hh + 128],
                            pts[p][hh][bi][:, F + 512 * b: F + 512 * b + w],
                            start=(n == 0), stop=(n == len(kcs) - 1),
                            skip_group_check=True)
                    # approx-fast reciprocal cannot read PSUM; bounce the
                    # replicated denominators through SBUF
                    rec = rcp.tile([64, 1024], F32, tag="rec")
                    nc.vector.tensor_copy(rec[:, 0:512], avb[drow:drow + 64, :])
                    nc.vector.reciprocal_approx_fast(rec[:, 512:1024], rec[:, 0:512])
                    nc.vector.tensor_tensor(XT[p][hr, 512 * b:512 * (b + 1)],
                                            avb[orow:orow + 64, :], rec[:, 512:1024],
                                            MULT)

            # ---- output projection for q-tiles [i0, i1) ----
            def out_proj(i0, i1):
                for i in range(i0, i1):
                    po = pjp.tile([P, 512], F32, tag="pj")
                    for ch in range(4):
                        nc.tensor.matmul(po[:], XT[ch][:, P * i:P * (i + 1)],
                                         wo[:, 512 * ch:512 * (ch + 1)],
                                         start=(ch == 0), stop=(ch == 3))
                    ob = ost.tile([P, D], F32, tag="ob")
                    nc.vector.tensor_tensor(ob[:], po[:], bo[:], ADD)
                    nc.sync.dma_start(out_d[P * i:P * (i + 1), :], ob[:])

            # ---- pipelined emission ----
            # Per slot: score bins first (they feed ACT, the pacer), then
            # next pair's projections, then AV work (naturally back-loaded).
            project(0)
            for p in range(NPAIR):
                for bi in range(NBIN):
                    score_bin(p, bi)
                if p < NPAIR - 1:
                    project(p + 1)
                if p > 0:
                    attention_av(2 * (p - 1) + 1)   # h1 of previous pair
                attention_av(2 * p)                 # h0 of this pair
            # drain: h1 of pair 3, with the output projection split so its
            # first half overlaps the second AV bank
            attention_av(7, banks=(0,))
            out_proj(0, 4)
            attention_av(7, banks=(1,))
            out_proj(4, 8)

    nc.compile()
    _cache["nc"] = nc
    return nc


def _host_prep(query, key, value, Wq, bq, Wk, bk, Wv, bv, Wo, bo):
    """Build the 8 per-core input maps (all device-side layouts)."""
    def stack_pairs(W):
        # [H,D,A] -> [128, 16*128]: col block (4ch+p) = rows 128ch of
        # [W_2p|W_2p+1] (ch-major so a per-ch DMA chunk is contiguous)
        blocks = []
        for ch in range(4):
            for p in range(NPAIR):
                Wp = np.concatenate([W[2 * p], W[2 * p + 1]], axis=1)  # [512, 128]
                blocks.append(Wp[P * ch:P * (ch + 1), :])
        return np.stack(blocks, 1).reshape(P, -1).astype(BF)  # [128,2048]

    wq_h, wk_h, wv_h = stack_pairs(Wq), stack_pairs(Wk), stack_pairs(Wv)
    wo_h = np.stack([Wo[P * ch:P * (ch + 1), :] for ch in range(4)], 1)
    wo_h = wo_h.reshape(P, -1).astype(BF)  # [128, 4*512]

    def stack_bias(b, scale=1.0):
        cols = [np.concatenate([b[2 * p], b[2 * p + 1]]) * scale for p in range(NPAIR)]
        return np.stack(cols, 1).astype(np.float32)  # [128, 4]

    bq_h = stack_bias(bq, 0.125)
    bk_h, bv_h = stack_bias(bk), stack_bias(bv)
    bo_h = np.broadcast_to(bo[None, :], (P, D)).astype(BF)
    id_h = np.eye(P, dtype=BF)
    kl = np.arange(P)[:, None]
    ql = np.arange(P)[None, :]
    tril_strict = (kl > ql).astype(BF)

    def chunked_T(x, scale=1.0):
        # [S', D] -> [128, 4*S'] with col block ch = rows 128ch of x.T
        xT = np.ascontiguousarray(x.T) * scale  # [512, S']
        return xT.reshape(4, P, -1).transpose(1, 0, 2).reshape(P, -1).astype(BF)

    in_maps = []
    for c in range(8):
        b, pair = c // 2, c % 2
        sel = np.concatenate(
            [np.arange(P * (2 * i + pair), P * (2 * i + pair) + P) for i in range(NQT)])
        m = {
            "qT8": chunked_T(query[b][sel], 0.125),
            "kT": chunked_T(key[b]),
            "vT": chunked_T(value[b]),
            "wq": wq_h, "wk": wk_h, "wv": wv_h, "wo": wo_h,
            "bq8": bq_h, "bk": bk_h, "bv": bv_h, "bo128": bo_h,
            "ident": id_h,
            "maskE": tril_strict if pair == 0 else np.zeros((P, P), BF),
            "maskO": np.ones((P, P), BF) if pair == 0 else tril_strict,
        }
        in_maps.append(m)
    return in_maps


def kernel(query, key, value, Wq, bq, Wk, bk, Wv, bv, Wo, bo):
    from concourse.bass_utils import run_bass_kernel_spmd

    args = [np.asarray(a, dtype=np.float32) for a in
            (query, key, value, Wq, bq, Wk, bk, Wv, bv, Wo, bo)]
    query, key, value, Wq, bq, Wk, bk, Wv, bv, Wo, bo = args

    nc = _build()
    in_maps = _host_prep(*args)
    res = run_bass_kernel_spmd(nc, in_maps, list(range(8)))

    out = np.empty((B, S, D), np.float32)
    for c in range(8):
        b, pair = c // 2, c % 2
        o = res.results[c]["out"]
        for i in range(NQT):
            g = 2 * i + pair
            out[b, P * g:P * (g + 1), :] = o[P * i:P * (i + 1), :]

    # q = S-1 attends to nothing -> reference softmax is uniform over all keys
    for b in range(B):
        vm = value[b].mean(0)
        x = np.concatenate([vm @ Wv[h] + bv[h] for h in range(H)])
        out[b, S - 1, :] = x @ Wo + bo
    return out
